# revision 1
# baseline (speedup 1.0000x reference)
"""Paged KV-cache scatter write (nn_KvPageCache) for 8 Trainium2 NeuronCores.

Semantics (matches jax reference, incl. last-wins on duplicate (page,slot)):
    out = kv_pages.copy()
    out[t_pages[i], t_slots[i], 0::2, :] = new_k[i]   # k -> even kv-head slots
    out[t_pages[i], t_slots[i], 1::2, :] = new_v[i]   # v -> odd  kv-head slots

Strategy:
  - Shard the page axis across the 8 cores: 512 contiguous pages / core
    (contiguous 67 MB shard -> large line-rate DMA descriptors, unlike the
    kv-head sharding which fragments every row into 1 KB runs).
  - Host side: drop out-of-range tokens, dedupe (page,slot) keeping the
    LAST occurrence (reference scatter is last-wins), route each token to
    the core owning its page, pad per-core token lists to a common length
    with idempotent repeats (SPMD needs one compiled program).
  - Device side per core: one big DRAM->DRAM copy of the shard, then an
    indirect-DMA scatter: the k/v rows of 128 tokens are staged interleaved
    in SBUF as one [128, 2048] f32 tile (one 8 KB contiguous row per token)
    and scattered to row offsets read from SBUF.
"""

import sys

if "/opt/trn_rl_repo" not in sys.path:
    sys.path.insert(0, "/opt/trn_rl_repo")

import numpy as np

NUM_PAGES = 4096
PAGE_SIZE = 16
KV_HEADS = 8
HEAD_SIZE = 128
N_CORES = 8
PAGES_PER_CORE = NUM_PAGES // N_CORES          # 512
R = PAGES_PER_CORE * PAGE_SIZE                 # 8192 rows per core shard
ROW = 2 * KV_HEADS * HEAD_SIZE                 # 2048 f32 = 8 KB per (page,slot)
HALF = KV_HEADS * HEAD_SIZE                    # 1024 f32 (k or v part of a row)

_cache: dict = {}


def build_program(n_pad: int, r: int = R, row: int = ROW, head: int = HEAD_SIZE,
                  n_copy_chunks: int = 16, tile_chunk_deps=None):
    """Build + compile the per-core Bass program (SPMD: same program, 8 cores).

    Tensors (per core):
      kv  [r, row]  f32  in   - the core's kv_pages shard, rows = page*16+slot
      upk [n_pad, row//2] f32 in - gathered new_k rows of this core's tokens
      upv [n_pad, row//2] f32 in
      idx [128, n_pad//128] i32 in - idx[p, t] = dest row of token t*128+p
      out [r, row]  f32  out
    """
    import concourse.bacc as bacc
    import concourse.bass as bass
    import concourse.tile as tile
    from concourse import mybir
    from concourse.tile import add_dep_helper

    assert n_pad % 128 == 0
    n_tiles = n_pad // 128
    heads = (row // 2) // head

    nc = bacc.Bacc("TRN2", target_bir_lowering=False, debug=False)
    f32, i32 = mybir.dt.float32, mybir.dt.int32

    kv = nc.dram_tensor("kv", [r, row], f32, kind="ExternalInput").ap()
    upk = nc.dram_tensor("upk", [n_pad, row // 2], f32, kind="ExternalInput").ap()
    upv = nc.dram_tensor("upv", [n_pad, row // 2], f32, kind="ExternalInput").ap()
    idx = nc.dram_tensor("idx", [128, n_tiles], i32, kind="ExternalInput").ap()
    out = nc.dram_tensor("out", [r, row], f32, kind="ExternalOutput").ap()

    with tile.TileContext(nc) as tc:
        with (
            tc.tile_pool(name="upd", bufs=max(2, min(4, n_tiles))) as upd_pool,
            tc.tile_pool(name="idxp", bufs=1) as idx_pool,
        ):
            # ---- bulk copy kv -> out, in big contiguous chunks ----
            assert r % n_copy_chunks == 0
            rows_per = r // n_copy_chunks
            copies = []
            for j in range(n_copy_chunks):
                ci = nc.sync.dma_start(
                    out=out[j * rows_per:(j + 1) * rows_per, :],
                    in_=kv[j * rows_per:(j + 1) * rows_per, :],
                )
                copies.append(ci)

            # ---- all scatter indices in one small DMA ----
            idx_t = idx_pool.tile([128, n_tiles], i32)
            nc.sync.dma_start(out=idx_t[:], in_=idx[:, :])

            # ---- scatter: stage 128 interleaved rows, indirect-write 8KB each ----
            for t in range(n_tiles):
                upd = upd_pool.tile([128, row], f32)
                u4 = upd[:].rearrange("p (h two d) -> p h two d", two=2, d=head)
                nc.sync.dma_start(
                    out=u4[:, :, 0, :],
                    in_=upk[t * 128:(t + 1) * 128, :].rearrange(
                        "p (h d) -> p h d", d=head),
                )
                nc.sync.dma_start(
                    out=u4[:, :, 1, :],
                    in_=upv[t * 128:(t + 1) * 128, :].rearrange(
                        "p (h d) -> p h d", d=head),
                )
                sc = nc.gpsimd.indirect_dma_start(
                    out=out[:, :],
                    out_offset=bass.IndirectOffsetOnAxis(ap=idx_t[:, t:t + 1], axis=0),
                    in_=upd[:],
                    in_offset=None,
                )
                # scatter rows were just copied by the bulk copy; enforce WAW
                # order. With tokens sorted by dest row, tile t only touches
                # rows in tile_chunk_deps[t] -> overlap scatter w/ copy tail.
                deps = (range(n_copy_chunks) if tile_chunk_deps is None
                        else tile_chunk_deps[t])
                for j in deps:
                    add_dep_helper(sc.ins, copies[j].ins, reason="scatter-after-copy")

    nc.compile()
    return nc


def _prep_inputs(kv_pages, new_k, new_v, t_pages, t_slots):
    """Host-side shard prep. Returns (n_pad, in_maps)."""
    kvf = np.ascontiguousarray(kv_pages, dtype=np.float32).reshape(
        NUM_PAGES * PAGE_SIZE, ROW)
    nk = np.ascontiguousarray(new_k, dtype=np.float32).reshape(-1, HALF)
    nv = np.ascontiguousarray(new_v, dtype=np.float32).reshape(-1, HALF)
    tp = np.asarray(t_pages).astype(np.int64)
    ts = np.asarray(t_slots).astype(np.int64)
    n = tp.shape[0]

    # drop-mode semantics: out-of-range tokens are ignored
    valid = (tp >= 0) & (tp < NUM_PAGES) & (ts >= 0) & (ts < PAGE_SIZE)
    order = np.arange(n)
    vidx = order[valid]
    vkey = (tp * PAGE_SIZE + ts)[valid]
    # keep LAST occurrence per (page,slot): sort by (key, order), take group tails
    perm = np.lexsort((vidx, vkey))
    sk = vkey[perm]
    tail = np.ones(len(sk), dtype=bool)
    if len(sk) > 1:
        tail[:-1] = sk[1:] != sk[:-1]
    keep = vidx[perm[tail]]                     # unique rows, last writer kept

    ktp = tp[keep]
    core = ktp // PAGES_PER_CORE
    local = (ktp % PAGES_PER_CORE) * PAGE_SIZE + ts[keep]

    counts = np.bincount(core, minlength=N_CORES)
    n_pad = max(128, int(-(-counts.max() // 128) * 128))
    n_tiles = n_pad // 128

    in_maps = []
    for c in range(N_CORES):
        sel = np.nonzero(core == c)[0]
        n_c = len(sel)
        if n_c == 0:
            # no tokens for this core: rewrite row 0 with its own (copied) data
            row0 = kvf[c * R].reshape(heads2 := 2 * KV_HEADS, HEAD_SIZE)
            upk_c = np.broadcast_to(row0[0::2].reshape(-1), (n_pad, HALF)).copy()
            upv_c = np.broadcast_to(row0[1::2].reshape(-1), (n_pad, HALF)).copy()
            loc_p = np.zeros(n_pad, dtype=np.int64)
        else:
            tok = keep[sel]
            loc = local[sel]
            o = np.argsort(loc)            # sort by dest row for chunk-local deps
            tok, loc = tok[o], loc[o]
            pad = n_pad - n_c
            tok_p = np.concatenate([tok, np.repeat(tok[-1:], pad)])
            loc_p = np.concatenate([loc, np.repeat(loc[-1:], pad)])
            upk_c = nk[tok_p]
            upv_c = nv[tok_p]
        idx_c = np.ascontiguousarray(
            loc_p.reshape(n_tiles, 128).T.astype(np.int32))
        in_maps.append({
            "kv": kvf[c * R:(c + 1) * R],
            "upk": np.ascontiguousarray(upk_c),
            "upv": np.ascontiguousarray(upv_c),
            "idx": idx_c,
        })
    # exact union over cores of each tile's dest-row range -> copy-chunk deps
    n_chunks = 16
    rows_per = R // n_chunks
    tile_chunk_deps = []
    for t in range(n_tiles):
        lo = min(int(im["idx"][:, t].min()) for im in in_maps)
        hi = max(int(im["idx"][:, t].max()) for im in in_maps)
        tile_chunk_deps.append(tuple(range(lo // rows_per, hi // rows_per + 1)))
    return n_pad, in_maps, tuple(tile_chunk_deps)


def kernel(kv_pages, new_k, new_v, t_pages, t_slots):
    from concourse.bass_utils import run_bass_kernel_spmd

    n_pad, in_maps, tile_chunk_deps = _prep_inputs(
        kv_pages, new_k, new_v, t_pages, t_slots)
    ckey = (n_pad, tile_chunk_deps)
    nc = _cache.get(ckey)
    if nc is None:
        nc = _cache[ckey] = build_program(n_pad, tile_chunk_deps=tile_chunk_deps)
    res = run_bass_kernel_spmd(nc, in_maps, core_ids=list(range(N_CORES))).results
    out = np.concatenate([res[c]["out"] for c in range(N_CORES)], axis=0)
    return out.reshape(NUM_PAGES, PAGE_SIZE, 2 * KV_HEADS, HEAD_SIZE)



# revision 3
# speedup vs baseline: 45.4274x; 45.4274x over previous
"""Paged KV-cache scatter write (nn_KvPageCache) for 8 Trainium2 NeuronCores.

Semantics (matches jax reference, incl. last-wins on duplicate (page,slot)):
    out = kv_pages.copy()
    out[t_pages[i], t_slots[i], 0::2, :] = new_k[i]   # k -> even kv-head slots
    out[t_pages[i], t_slots[i], 1::2, :] = new_v[i]   # v -> odd  kv-head slots

Strategy:
  - Shard the page axis across the 8 cores: 512 contiguous pages / core
    (contiguous 67 MB shard -> large line-rate DMA descriptors, unlike the
    kv-head sharding which fragments every row into 1 KB runs).
  - Host side: drop out-of-range tokens, dedupe (page,slot) keeping the
    LAST occurrence (reference scatter is last-wins), route each token to
    the core owning its page, sort by destination row, pad per-core token
    lists to a common length with idempotent repeats (SPMD needs one
    compiled program), and interleave each token's k/v heads into one
    contiguous 8 KB row (the destination row layout) so the device stages
    it with 8 KB line-rate descriptors.
  - Device side per core, issue order chosen so everything overlaps the
    bulk copy (HWDGE queue drains in FIFO order — anything issued after
    the 64 MB copy would wait ~215 us for it):
      1. scatter-index tile + all update tiles -> SBUF (tiny + 8 MB)
      2. bulk copy kv -> out in big contiguous chunks (64 MB)
      3. per tile of 128 tokens: indirect-DMA scatter of 8 KB rows
         (SWDGE queue, runs concurrently with the copy), WAW-ordered
         after only the copy chunks its sorted rows fall into.
"""

import sys

if "/opt/trn_rl_repo" not in sys.path:
    sys.path.insert(0, "/opt/trn_rl_repo")

import numpy as np

NUM_PAGES = 4096
PAGE_SIZE = 16
KV_HEADS = 8
HEAD_SIZE = 128
N_CORES = 8
PAGES_PER_CORE = NUM_PAGES // N_CORES          # 512
R = PAGES_PER_CORE * PAGE_SIZE                 # 8192 rows per core shard
ROW = 2 * KV_HEADS * HEAD_SIZE                 # 2048 f32 = 8 KB per (page,slot)
HALF = KV_HEADS * HEAD_SIZE                    # 1024 f32 (k or v part of a row)
MAX_UPD_TILES = 16                             # SBUF cap: 16 x 1 MB staged tiles

_cache: dict = {}


def build_program(n_pad: int, r: int = R, row: int = ROW,
                  n_copy_chunks: int = 16, tile_chunk_deps=None):
    """Build + compile the per-core Bass program (SPMD: same program, 8 cores).

    Tensors (per core):
      kv  [r, row]  f32  in  - the core's kv_pages shard, rows = page*16+slot
      upd [n_pad, row] f32 in - this core's token rows, k/v pre-interleaved
      idx [128, n_pad//128] i32 in - idx[p, t] = dest row of token t*128+p
      out [r, row]  f32  out
    """
    import concourse.bacc as bacc
    import concourse.bass as bass
    import concourse.tile as tile
    from concourse import mybir
    from concourse.tile import add_dep_helper

    assert n_pad % 128 == 0
    n_tiles = n_pad // 128

    nc = bacc.Bacc("TRN2", target_bir_lowering=False, debug=False)
    f32, i32 = mybir.dt.float32, mybir.dt.int32

    kv = nc.dram_tensor("kv", [r, row], f32, kind="ExternalInput").ap()
    upd = nc.dram_tensor("upd", [n_pad, row], f32, kind="ExternalInput").ap()
    idx = nc.dram_tensor("idx", [128, n_tiles], i32, kind="ExternalInput").ap()
    out = nc.dram_tensor("out", [r, row], f32, kind="ExternalOutput").ap()

    with tile.TileContext(nc) as tc:
        with (
            tc.tile_pool(name="upd", bufs=min(n_tiles, MAX_UPD_TILES)) as upd_pool,
            tc.tile_pool(name="idxp", bufs=1) as idx_pool,
        ):
            # ---- scatter indices + update rows first: they are small and
            # the scatters need them; issued after the copy they would sit
            # behind 64 MB in the HWDGE FIFO.
            idx_t = idx_pool.tile([128, n_tiles], i32)
            nc.sync.dma_start(out=idx_t[:], in_=idx[:, :])

            # Stage up to MAX_UPD_TILES before the copy. Staging a tile whose
            # pool buffer is still owned by a pending scatter would stall the
            # in-order SP queue (and every copy behind it) on that scatter,
            # which itself waits on copies -> deadlock. Overflow tiles are
            # staged after the copies are issued instead.
            n_pre = min(n_tiles, MAX_UPD_TILES)
            upd_tiles = []
            for t in range(n_pre):
                u = upd_pool.tile([128, row], f32)
                nc.sync.dma_start(out=u[:], in_=upd[t * 128:(t + 1) * 128, :])
                upd_tiles.append(u)

            # ---- bulk copy kv -> out, in big contiguous chunks ----
            assert r % n_copy_chunks == 0
            rows_per = r // n_copy_chunks
            copies = []
            for j in range(n_copy_chunks):
                ci = nc.sync.dma_start(
                    out=out[j * rows_per:(j + 1) * rows_per, :],
                    in_=kv[j * rows_per:(j + 1) * rows_per, :],
                )
                copies.append(ci)

            # ---- scatter: indirect-write 8 KB rows, overlapping the copy.
            # Scatter rows were just copied by the bulk copy; enforce WAW
            # order. With tokens sorted by dest row, tile t only touches
            # rows in tile_chunk_deps[t] -> runs as soon as those chunks
            # land instead of waiting for the whole copy.
            for t in range(n_tiles):
                if t >= n_pre:
                    u = upd_pool.tile([128, row], f32)
                    nc.sync.dma_start(out=u[:], in_=upd[t * 128:(t + 1) * 128, :])
                    upd_tiles.append(u)
                sc = nc.gpsimd.indirect_dma_start(
                    out=out[:, :],
                    out_offset=bass.IndirectOffsetOnAxis(ap=idx_t[:, t:t + 1], axis=0),
                    in_=upd_tiles[t][:],
                    in_offset=None,
                )
                deps = (range(n_copy_chunks) if tile_chunk_deps is None
                        else tile_chunk_deps[t])
                for j in deps:
                    add_dep_helper(sc.ins, copies[j].ins, reason="scatter-after-copy")

    nc.compile()
    return nc


def _prep_inputs(kv_pages, new_k, new_v, t_pages, t_slots):
    """Host-side shard prep. Returns (n_pad, in_maps, tile_chunk_deps)."""
    kvf = np.ascontiguousarray(kv_pages, dtype=np.float32).reshape(
        NUM_PAGES * PAGE_SIZE, ROW)
    nk = np.ascontiguousarray(new_k, dtype=np.float32).reshape(-1, KV_HEADS, HEAD_SIZE)
    nv = np.ascontiguousarray(new_v, dtype=np.float32).reshape(-1, KV_HEADS, HEAD_SIZE)
    tp = np.asarray(t_pages).astype(np.int64)
    ts = np.asarray(t_slots).astype(np.int64)
    n = tp.shape[0]

    # drop-mode semantics: out-of-range tokens are ignored
    valid = (tp >= 0) & (tp < NUM_PAGES) & (ts >= 0) & (ts < PAGE_SIZE)
    order = np.arange(n)
    vidx = order[valid]
    vkey = (tp * PAGE_SIZE + ts)[valid]
    # keep LAST occurrence per (page,slot): sort by (key, order), take group tails
    perm = np.lexsort((vidx, vkey))
    sk = vkey[perm]
    tail = np.ones(len(sk), dtype=bool)
    if len(sk) > 1:
        tail[:-1] = sk[1:] != sk[:-1]
    keep = vidx[perm[tail]]                     # unique rows, last writer kept

    ktp = tp[keep]
    core = ktp // PAGES_PER_CORE
    local = (ktp % PAGES_PER_CORE) * PAGE_SIZE + ts[keep]

    counts = np.bincount(core, minlength=N_CORES)
    n_pad = max(128, int(-(-counts.max() // 128) * 128))
    n_tiles = n_pad // 128

    in_maps = []
    for c in range(N_CORES):
        sel = np.nonzero(core == c)[0]
        n_c = len(sel)
        upd_c = np.empty((n_pad, KV_HEADS, 2, HEAD_SIZE), dtype=np.float32)
        if n_c == 0:
            # no tokens for this core: rewrite row 0 with its own (copied) data
            row0 = kvf[c * R].reshape(2 * KV_HEADS, HEAD_SIZE)
            upd_c[:, :, 0, :] = row0[0::2]
            upd_c[:, :, 1, :] = row0[1::2]
            loc_p = np.zeros(n_pad, dtype=np.int64)
        else:
            tok = keep[sel]
            loc = local[sel]
            o = np.argsort(loc)            # sort by dest row for chunk-local deps
            tok, loc = tok[o], loc[o]
            pad = n_pad - n_c
            tok_p = np.concatenate([tok, np.repeat(tok[-1:], pad)])
            loc_p = np.concatenate([loc, np.repeat(loc[-1:], pad)])
            upd_c[:, :, 0, :] = nk[tok_p]
            upd_c[:, :, 1, :] = nv[tok_p]
        idx_c = np.ascontiguousarray(
            loc_p.reshape(n_tiles, 128).T.astype(np.int32))
        in_maps.append({
            "kv": kvf[c * R:(c + 1) * R],
            "upd": upd_c.reshape(n_pad, ROW),
            "idx": idx_c,
        })
    # exact union over cores of each tile's dest-row range -> copy-chunk deps
    n_chunks = 16
    rows_per = R // n_chunks
    tile_chunk_deps = []
    for t in range(n_tiles):
        lo = min(int(im["idx"][:, t].min()) for im in in_maps)
        hi = max(int(im["idx"][:, t].max()) for im in in_maps)
        tile_chunk_deps.append(tuple(range(lo // rows_per, hi // rows_per + 1)))
    return n_pad, in_maps, tuple(tile_chunk_deps)


def kernel(kv_pages, new_k, new_v, t_pages, t_slots):
    from concourse.bass_utils import run_bass_kernel_spmd

    n_pad, in_maps, tile_chunk_deps = _prep_inputs(
        kv_pages, new_k, new_v, t_pages, t_slots)
    ckey = (n_pad, tile_chunk_deps)
    nc = _cache.get(ckey)
    if nc is None:
        nc = _cache[ckey] = build_program(n_pad, tile_chunk_deps=tile_chunk_deps)
    res = run_bass_kernel_spmd(nc, in_maps, core_ids=list(range(N_CORES))).results
    out = np.concatenate([res[c]["out"] for c in range(N_CORES)], axis=0)
    return out.reshape(NUM_PAGES, PAGE_SIZE, 2 * KV_HEADS, HEAD_SIZE)


# revision 4
# speedup vs baseline: 50.4514x; 1.1106x over previous
"""Paged KV-cache scatter write (nn_KvPageCache) for 8 Trainium2 NeuronCores.

Semantics (matches jax reference, incl. last-wins on duplicate (page,slot)):
    out = kv_pages.copy()
    out[t_pages[i], t_slots[i], 0::2, :] = new_k[i]   # k -> even kv-head slots
    out[t_pages[i], t_slots[i], 1::2, :] = new_v[i]   # v -> odd  kv-head slots

Strategy:
  - Shard the page axis across the 8 cores: 512 contiguous pages / core
    (contiguous 67 MB shard -> large line-rate DMA descriptors, unlike the
    kv-head sharding which fragments every row into 1 KB runs).
  - Host side: drop out-of-range tokens, dedupe (page,slot) keeping the
    LAST occurrence (reference scatter is last-wins), route each token to
    the core owning its page, sort by destination row, pad per-core token
    lists to a common length with idempotent repeats (SPMD needs one
    compiled program), and interleave each token's k/v heads into one
    contiguous 8 KB row (the destination row layout) so the device stages
    it with 8 KB line-rate descriptors.
  - Device side per core, issue order chosen so everything overlaps the
    bulk copy (HWDGE queue drains in FIFO order — anything issued after
    the 64 MB copy would wait ~215 us for it):
      1. scatter-index tile + all update tiles -> SBUF (tiny + 8 MB)
      2. bulk copy kv -> out in big contiguous chunks (64 MB)
      3. per tile of 128 tokens: indirect-DMA scatter of 8 KB rows
         (SWDGE queue, runs concurrently with the copy), WAW-ordered
         after only the copy chunks its sorted rows fall into.
"""

import sys

if "/opt/trn_rl_repo" not in sys.path:
    sys.path.insert(0, "/opt/trn_rl_repo")

import numpy as np

NUM_PAGES = 4096
PAGE_SIZE = 16
KV_HEADS = 8
HEAD_SIZE = 128
N_CORES = 8
PAGES_PER_CORE = NUM_PAGES // N_CORES          # 512
R = PAGES_PER_CORE * PAGE_SIZE                 # 8192 rows per core shard
ROW = 2 * KV_HEADS * HEAD_SIZE                 # 2048 f32 = 8 KB per (page,slot)
HALF = KV_HEADS * HEAD_SIZE                    # 1024 f32 (k or v part of a row)
MAX_UPD_TILES = 16                             # SBUF cap: 16 x 1 MB staged tiles

_cache: dict = {}


def build_program(n_pad: int, r: int = R, row: int = ROW,
                  n_copy_chunks: int = 16, tile_chunk_deps=None):
    """Build + compile the per-core Bass program (SPMD: same program, 8 cores).

    Tensors (per core):
      kv  [r, row]  f32  in  - the core's kv_pages shard, rows = page*16+slot
      upd [n_pad, row] f32 in - this core's token rows, k/v pre-interleaved
      idx [128, n_pad//128] i32 in - idx[p, t] = dest row of token t*128+p
      out [r, row]  f32  out
    """
    import concourse.bacc as bacc
    import concourse.bass as bass
    import concourse.tile as tile
    from concourse import mybir
    from concourse.tile import add_dep_helper

    assert n_pad % 128 == 0
    n_tiles = n_pad // 128

    nc = bacc.Bacc("TRN2", target_bir_lowering=False, debug=False)
    f32, i32 = mybir.dt.float32, mybir.dt.int32

    kv = nc.dram_tensor("kv", [r, row], f32, kind="ExternalInput").ap()
    upd = nc.dram_tensor("upd", [n_pad, row], f32, kind="ExternalInput").ap()
    idx = nc.dram_tensor("idx", [128, n_tiles], i32, kind="ExternalInput").ap()
    out = nc.dram_tensor("out", [r, row], f32, kind="ExternalOutput").ap()

    with tile.TileContext(nc) as tc:
        with (
            tc.tile_pool(name="upd", bufs=min(n_tiles, MAX_UPD_TILES)) as upd_pool,
            tc.tile_pool(name="idxp", bufs=1) as idx_pool,
        ):
            # ---- scatter indices + update rows first: they are small and
            # the scatters need them; issued after the copy they would sit
            # behind 64 MB in the HWDGE FIFO.
            idx_t = idx_pool.tile([128, n_tiles], i32)
            nc.sync.dma_start(out=idx_t[:], in_=idx[:, :])

            # Stage up to MAX_UPD_TILES before the copy. Staging a tile whose
            # pool buffer is still owned by a pending scatter would stall the
            # in-order SP queue (and every copy behind it) on that scatter,
            # which itself waits on copies -> deadlock. Overflow tiles are
            # staged after the copies are issued instead.
            n_pre = min(n_tiles, MAX_UPD_TILES)
            upd_tiles = []
            for t in range(n_pre):
                u = upd_pool.tile([128, row], f32)
                nc.sync.dma_start(out=u[:], in_=upd[t * 128:(t + 1) * 128, :])
                upd_tiles.append(u)

            # ---- bulk copy kv -> out, in big contiguous chunks ----
            assert r % n_copy_chunks == 0
            rows_per = r // n_copy_chunks
            copies = []
            for j in range(n_copy_chunks):
                ci = nc.sync.dma_start(
                    out=out[j * rows_per:(j + 1) * rows_per, :],
                    in_=kv[j * rows_per:(j + 1) * rows_per, :],
                )
                copies.append(ci)

            # ---- scatter: indirect-write 8 KB rows, overlapping the copy.
            # Scatter rows were just copied by the bulk copy; enforce WAW
            # order. With tokens sorted by dest row, tile t only touches
            # rows in tile_chunk_deps[t] -> runs as soon as those chunks
            # land instead of waiting for the whole copy.
            for t in range(n_tiles):
                if t >= n_pre:
                    u = upd_pool.tile([128, row], f32)
                    nc.sync.dma_start(out=u[:], in_=upd[t * 128:(t + 1) * 128, :])
                    upd_tiles.append(u)
                sc = nc.gpsimd.indirect_dma_start(
                    out=out[:, :],
                    out_offset=bass.IndirectOffsetOnAxis(ap=idx_t[:, t:t + 1], axis=0),
                    in_=upd_tiles[t][:],
                    in_offset=None,
                )
                deps = (range(n_copy_chunks) if tile_chunk_deps is None
                        else tile_chunk_deps[t])
                for j in deps:
                    add_dep_helper(sc.ins, copies[j].ins, reason="scatter-after-copy")
                # The framework auto-adds a whole-tensor WAW dep on `out`,
                # making every scatter wait for ALL copy chunks (serializing
                # scatter behind the full 64 MB copy). The sorted-token chunk
                # ranges above are the exact WAW set; drop the rest.
                keep = set(deps)
                for j in range(n_copy_chunks):
                    if j not in keep:
                        sc.ins.try_remove_dependency(copies[j].ins.name)

    nc.compile()
    return nc


def _prep_inputs(kv_pages, new_k, new_v, t_pages, t_slots):
    """Host-side shard prep. Returns (n_pad, in_maps, tile_chunk_deps)."""
    kvf = np.ascontiguousarray(kv_pages, dtype=np.float32).reshape(
        NUM_PAGES * PAGE_SIZE, ROW)
    nk = np.ascontiguousarray(new_k, dtype=np.float32).reshape(-1, KV_HEADS, HEAD_SIZE)
    nv = np.ascontiguousarray(new_v, dtype=np.float32).reshape(-1, KV_HEADS, HEAD_SIZE)
    tp = np.asarray(t_pages).astype(np.int64)
    ts = np.asarray(t_slots).astype(np.int64)
    n = tp.shape[0]

    # drop-mode semantics: out-of-range tokens are ignored
    valid = (tp >= 0) & (tp < NUM_PAGES) & (ts >= 0) & (ts < PAGE_SIZE)
    order = np.arange(n)
    vidx = order[valid]
    vkey = (tp * PAGE_SIZE + ts)[valid]
    # keep LAST occurrence per (page,slot): sort by (key, order), take group tails
    perm = np.lexsort((vidx, vkey))
    sk = vkey[perm]
    tail = np.ones(len(sk), dtype=bool)
    if len(sk) > 1:
        tail[:-1] = sk[1:] != sk[:-1]
    keep = vidx[perm[tail]]                     # unique rows, last writer kept

    ktp = tp[keep]
    core = ktp // PAGES_PER_CORE
    local = (ktp % PAGES_PER_CORE) * PAGE_SIZE + ts[keep]

    counts = np.bincount(core, minlength=N_CORES)
    n_pad = max(128, int(-(-counts.max() // 128) * 128))
    n_tiles = n_pad // 128

    in_maps = []
    for c in range(N_CORES):
        sel = np.nonzero(core == c)[0]
        n_c = len(sel)
        upd_c = np.empty((n_pad, KV_HEADS, 2, HEAD_SIZE), dtype=np.float32)
        if n_c == 0:
            # no tokens for this core: rewrite row 0 with its own (copied) data
            row0 = kvf[c * R].reshape(2 * KV_HEADS, HEAD_SIZE)
            upd_c[:, :, 0, :] = row0[0::2]
            upd_c[:, :, 1, :] = row0[1::2]
            loc_p = np.zeros(n_pad, dtype=np.int64)
        else:
            tok = keep[sel]
            loc = local[sel]
            o = np.argsort(loc)            # sort by dest row for chunk-local deps
            tok, loc = tok[o], loc[o]
            pad = n_pad - n_c
            tok_p = np.concatenate([tok, np.repeat(tok[-1:], pad)])
            loc_p = np.concatenate([loc, np.repeat(loc[-1:], pad)])
            upd_c[:, :, 0, :] = nk[tok_p]
            upd_c[:, :, 1, :] = nv[tok_p]
        idx_c = np.ascontiguousarray(
            loc_p.reshape(n_tiles, 128).T.astype(np.int32))
        in_maps.append({
            "kv": kvf[c * R:(c + 1) * R],
            "upd": upd_c.reshape(n_pad, ROW),
            "idx": idx_c,
        })
    # exact union over cores of each tile's dest-row range -> copy-chunk deps
    n_chunks = 16
    rows_per = R // n_chunks
    tile_chunk_deps = []
    for t in range(n_tiles):
        lo = min(int(im["idx"][:, t].min()) for im in in_maps)
        hi = max(int(im["idx"][:, t].max()) for im in in_maps)
        tile_chunk_deps.append(tuple(range(lo // rows_per, hi // rows_per + 1)))
    return n_pad, in_maps, tuple(tile_chunk_deps)


def kernel(kv_pages, new_k, new_v, t_pages, t_slots):
    from concourse.bass_utils import run_bass_kernel_spmd

    n_pad, in_maps, tile_chunk_deps = _prep_inputs(
        kv_pages, new_k, new_v, t_pages, t_slots)
    ckey = (n_pad, tile_chunk_deps)
    nc = _cache.get(ckey)
    if nc is None:
        nc = _cache[ckey] = build_program(n_pad, tile_chunk_deps=tile_chunk_deps)
    res = run_bass_kernel_spmd(nc, in_maps, core_ids=list(range(N_CORES))).results
    out = np.concatenate([res[c]["out"] for c in range(N_CORES)], axis=0)
    return out.reshape(NUM_PAGES, PAGE_SIZE, 2 * KV_HEADS, HEAD_SIZE)


# revision 8
# speedup vs baseline: 53.5089x; 1.0606x over previous
"""Paged KV-cache scatter write (nn_KvPageCache) for 8 Trainium2 NeuronCores.

Semantics (matches jax reference, incl. last-wins on duplicate (page,slot)):
    out = kv_pages.copy()
    out[t_pages[i], t_slots[i], 0::2, :] = new_k[i]   # k -> even kv-head slots
    out[t_pages[i], t_slots[i], 1::2, :] = new_v[i]   # v -> odd  kv-head slots

Strategy:
  - Shard the page axis across the 8 cores: 512 contiguous pages / core
    (contiguous 67 MB shard -> large line-rate DMA descriptors, unlike the
    kv-head sharding which fragments every row into 1 KB runs).
  - Host side: drop out-of-range tokens, dedupe (page,slot) keeping the
    LAST occurrence (reference scatter is last-wins), route each token to
    the core owning its page, sort by destination row, pad per-core token
    lists to a common length with idempotent repeats (SPMD needs one
    compiled program), and interleave each token's k/v heads into one
    contiguous 8 KB row (the destination row layout) so the device stages
    it with 8 KB line-rate descriptors.
  - Device side per core, issue order chosen so everything overlaps the
    bulk copy (HWDGE queue drains in FIFO order — anything issued after
    the 64 MB copy would wait ~215 us for it):
      1. scatter-index tile + all update tiles -> SBUF (tiny + 8 MB)
      2. bulk copy kv -> out in big contiguous chunks (64 MB)
      3. per tile of 128 tokens: indirect-DMA scatter of 8 KB rows
         (SWDGE queue, runs concurrently with the copy), WAW-ordered
         after only the copy chunks its sorted rows fall into.
"""

import sys

if "/opt/trn_rl_repo" not in sys.path:
    sys.path.insert(0, "/opt/trn_rl_repo")

import numpy as np

NUM_PAGES = 4096
PAGE_SIZE = 16
KV_HEADS = 8
HEAD_SIZE = 128
N_CORES = 8
PAGES_PER_CORE = NUM_PAGES // N_CORES          # 512
R = PAGES_PER_CORE * PAGE_SIZE                 # 8192 rows per core shard
ROW = 2 * KV_HEADS * HEAD_SIZE                 # 2048 f32 = 8 KB per (page,slot)
HALF = KV_HEADS * HEAD_SIZE                    # 1024 f32 (k or v part of a row)
MAX_UPD_TILES = 16                             # SBUF cap: 16 x 1 MB staged tiles

_cache: dict = {}


def default_chunk_bounds(r: int = R):
    """Copy-chunk row ranges: big chunks early, small chunks at the end.

    The last scatter tiles depend on the last chunks; small tail chunks
    stagger their completions so the final scatters pipeline instead of
    bunching behind one 8 MB chunk. Keeping the total DMA count modest
    matters: >25-ish outstanding DMAs on the queue degrades per-descriptor
    throughput sharply (measured: 32 uniform chunks ran 1.7x slower).
    """
    big = r * 6 // 8
    bounds = [(i * (big // 6), (i + 1) * (big // 6)) for i in range(6)]
    small = (r - big) // 8
    bounds += [(big + i * small, big + (i + 1) * small) for i in range(8)]
    return tuple(bounds)


def build_program(n_pad: int, r: int = R, row: int = ROW,
                  n_copy_chunks: int | None = None, tile_chunk_deps=None,
                  chunk_bounds=None):
    """Build + compile the per-core Bass program (SPMD: same program, 8 cores).

    Tensors (per core):
      kv  [r, row]  f32  in  - the core's kv_pages shard, rows = page*16+slot
      upd [n_pad, row] f32 in - this core's token rows, k/v pre-interleaved
      idx [128, n_pad//128] i32 in - idx[p, t] = dest row of token t*128+p
      out [r, row]  f32  out
    """
    import concourse.bacc as bacc
    import concourse.bass as bass
    import concourse.tile as tile
    from concourse import mybir
    from concourse.tile import add_dep_helper

    assert n_pad % 128 == 0
    n_tiles = n_pad // 128

    nc = bacc.Bacc("TRN2", target_bir_lowering=False, debug=False)
    f32, i32 = mybir.dt.float32, mybir.dt.int32

    kv = nc.dram_tensor("kv", [r, row], f32, kind="ExternalInput").ap()
    upd = nc.dram_tensor("upd", [n_pad, row], f32, kind="ExternalInput").ap()
    idx = nc.dram_tensor("idx", [128, n_tiles], i32, kind="ExternalInput").ap()
    out = nc.dram_tensor("out", [r, row], f32, kind="ExternalOutput").ap()

    with tile.TileContext(nc) as tc:
        with (
            tc.tile_pool(name="upd", bufs=min(n_tiles, MAX_UPD_TILES)) as upd_pool,
            tc.tile_pool(name="idxp", bufs=1) as idx_pool,
        ):
            # ---- scatter indices + update rows first: they are small and
            # the scatters need them; issued after the copy they would sit
            # behind 64 MB in the HWDGE FIFO.
            idx_t = idx_pool.tile([128, n_tiles], i32)
            nc.sync.dma_start(out=idx_t[:], in_=idx[:, :])

            # Stage up to MAX_UPD_TILES before the copy. Staging a tile whose
            # pool buffer is still owned by a pending scatter would stall the
            # in-order SP queue (and every copy behind it) on that scatter,
            # which itself waits on copies -> deadlock. Overflow tiles are
            # staged after the copies are issued instead.
            n_pre = min(n_tiles, MAX_UPD_TILES)
            upd_tiles = []
            for t in range(n_pre):
                u = upd_pool.tile([128, row], f32)
                nc.sync.dma_start(out=u[:], in_=upd[t * 128:(t + 1) * 128, :])
                upd_tiles.append(u)

            # ---- bulk copy kv -> out, in big contiguous chunks ----
            if chunk_bounds is None:
                if n_copy_chunks is not None:
                    assert r % n_copy_chunks == 0
                    rows_per = r // n_copy_chunks
                    chunk_bounds = tuple(
                        (j * rows_per, (j + 1) * rows_per)
                        for j in range(n_copy_chunks))
                else:
                    chunk_bounds = default_chunk_bounds(r)
            copies = []
            for (a, b) in chunk_bounds:
                ci = nc.sync.dma_start(out=out[a:b, :], in_=kv[a:b, :])
                copies.append(ci)

            # ---- scatter: indirect-write 8 KB rows, overlapping the copy.
            # Scatter rows were just copied by the bulk copy; enforce WAW
            # order. With tokens sorted by dest row, tile t only touches
            # rows in tile_chunk_deps[t] -> runs as soon as those chunks
            # land instead of waiting for the whole copy.
            for t in range(n_tiles):
                if t >= n_pre:
                    u = upd_pool.tile([128, row], f32)
                    nc.sync.dma_start(out=u[:], in_=upd[t * 128:(t + 1) * 128, :])
                    upd_tiles.append(u)
                sc = nc.gpsimd.indirect_dma_start(
                    out=out[:, :],
                    out_offset=bass.IndirectOffsetOnAxis(ap=idx_t[:, t:t + 1], axis=0),
                    in_=upd_tiles[t][:],
                    in_offset=None,
                )
                deps = (range(len(copies)) if tile_chunk_deps is None
                        else tile_chunk_deps[t])
                for j in deps:
                    add_dep_helper(sc.ins, copies[j].ins, reason="scatter-after-copy")
                # The framework auto-adds a whole-tensor WAW dep on `out`,
                # making every scatter wait for ALL copy chunks (serializing
                # scatter behind the full 64 MB copy). The sorted-token chunk
                # ranges above are the exact WAW set; drop the rest.
                keep = set(deps)
                for j in range(len(copies)):
                    if j not in keep:
                        sc.ins.try_remove_dependency(copies[j].ins.name)

    nc.compile()
    return nc


def _prep_inputs(kv_pages, new_k, new_v, t_pages, t_slots):
    """Host-side shard prep. Returns (n_pad, in_maps, tile_chunk_deps)."""
    kvf = np.ascontiguousarray(kv_pages, dtype=np.float32).reshape(
        NUM_PAGES * PAGE_SIZE, ROW)
    nk = np.ascontiguousarray(new_k, dtype=np.float32).reshape(-1, KV_HEADS, HEAD_SIZE)
    nv = np.ascontiguousarray(new_v, dtype=np.float32).reshape(-1, KV_HEADS, HEAD_SIZE)
    tp = np.asarray(t_pages).astype(np.int64)
    ts = np.asarray(t_slots).astype(np.int64)
    n = tp.shape[0]

    # drop-mode semantics: out-of-range tokens are ignored
    valid = (tp >= 0) & (tp < NUM_PAGES) & (ts >= 0) & (ts < PAGE_SIZE)
    order = np.arange(n)
    vidx = order[valid]
    vkey = (tp * PAGE_SIZE + ts)[valid]
    # keep LAST occurrence per (page,slot): sort by (key, order), take group tails
    perm = np.lexsort((vidx, vkey))
    sk = vkey[perm]
    tail = np.ones(len(sk), dtype=bool)
    if len(sk) > 1:
        tail[:-1] = sk[1:] != sk[:-1]
    keep = vidx[perm[tail]]                     # unique rows, last writer kept

    ktp = tp[keep]
    core = ktp // PAGES_PER_CORE
    local = (ktp % PAGES_PER_CORE) * PAGE_SIZE + ts[keep]

    counts = np.bincount(core, minlength=N_CORES)
    n_pad = max(128, int(-(-counts.max() // 128) * 128))
    n_tiles = n_pad // 128

    in_maps = []
    for c in range(N_CORES):
        sel = np.nonzero(core == c)[0]
        n_c = len(sel)
        upd_c = np.empty((n_pad, KV_HEADS, 2, HEAD_SIZE), dtype=np.float32)
        if n_c == 0:
            # no tokens for this core: rewrite row 0 with its own (copied) data
            row0 = kvf[c * R].reshape(2 * KV_HEADS, HEAD_SIZE)
            upd_c[:, :, 0, :] = row0[0::2]
            upd_c[:, :, 1, :] = row0[1::2]
            loc_p = np.zeros(n_pad, dtype=np.int64)
        else:
            tok = keep[sel]
            loc = local[sel]
            o = np.argsort(loc)            # sort by dest row for chunk-local deps
            tok, loc = tok[o], loc[o]
            pad = n_pad - n_c
            tok_p = np.concatenate([tok, np.repeat(tok[-1:], pad)])
            loc_p = np.concatenate([loc, np.repeat(loc[-1:], pad)])
            upd_c[:, :, 0, :] = nk[tok_p]
            upd_c[:, :, 1, :] = nv[tok_p]
        idx_c = np.ascontiguousarray(
            loc_p.reshape(n_tiles, 128).T.astype(np.int32))
        in_maps.append({
            "kv": kvf[c * R:(c + 1) * R],
            "upd": upd_c.reshape(n_pad, ROW),
            "idx": idx_c,
        })
    # exact union over cores of each tile's dest-row range -> copy-chunk deps
    bounds = default_chunk_bounds(R)
    tile_chunk_deps = []
    for t in range(n_tiles):
        lo = min(int(im["idx"][:, t].min()) for im in in_maps)
        hi = max(int(im["idx"][:, t].max()) for im in in_maps)
        tile_chunk_deps.append(tuple(
            j for j, (a, b) in enumerate(bounds) if b > lo and a <= hi))
    return n_pad, in_maps, tuple(tile_chunk_deps)


def kernel(kv_pages, new_k, new_v, t_pages, t_slots):
    from concourse.bass_utils import run_bass_kernel_spmd

    n_pad, in_maps, tile_chunk_deps = _prep_inputs(
        kv_pages, new_k, new_v, t_pages, t_slots)
    ckey = (n_pad, tile_chunk_deps)
    nc = _cache.get(ckey)
    if nc is None:
        nc = _cache[ckey] = build_program(n_pad, tile_chunk_deps=tile_chunk_deps)
    res = run_bass_kernel_spmd(nc, in_maps, core_ids=list(range(N_CORES))).results
    out = np.concatenate([res[c]["out"] for c in range(N_CORES)], axis=0)
    return out.reshape(NUM_PAGES, PAGE_SIZE, 2 * KV_HEADS, HEAD_SIZE)
